# revision 1
# baseline (speedup 1.0000x reference)
"""ChebNet GNN kernel for nn_Decimation_25142738551433.

kernel(**inputs) -> [128, 10] float32 log-softmax output.

The spectral propagation prop(y) = -D^-1/2 A D^-1/2 y is restructured as
per-node scaling (z = dinv*y) + an unweighted gather-sum over the fixed
edge list, evaluated as a CSR sparse-matrix product so the 39 sequential
Chebyshev propagations run at memory speed. Inputs are taken full-size;
all shapes below are hardcoded for this problem instance.
"""
import numpy as np

N = 100000
E = 1600000
F_IN = 128
HID = 64
K = 14
NUM_LAYERS = 3
NUM_GRAPHS = 128
NUM_CLASSES = 10

try:
    import scipy.sparse as sp
    from scipy.sparse import _sparsetools
    _HAVE_SCIPY = True
except Exception:
    _HAVE_SCIPY = False


def kernel(x, edge_index, batch, W1, theta1, b1, Ws, thetas, bs,
           lin1_w, lin1_b, lin2_w, lin2_b):
    x = np.asarray(x, np.float32)
    edge_index = np.asarray(edge_index)
    batch = np.asarray(batch).astype(np.int64)
    W1 = np.asarray(W1, np.float32)
    theta1 = np.asarray(theta1, np.float32)
    b1 = np.asarray(b1, np.float32)
    Ws = np.asarray(Ws, np.float32)
    thetas = np.asarray(thetas, np.float32)
    bs = np.asarray(bs, np.float32)
    lin1_w = np.asarray(lin1_w, np.float32)
    lin1_b = np.asarray(lin1_b, np.float32)
    lin2_w = np.asarray(lin2_w, np.float32)
    lin2_b = np.asarray(lin2_b, np.float32)

    row = edge_index[0].astype(np.int64)
    col = edge_index[1].astype(np.int64)
    n = x.shape[0]

    deg = np.bincount(row, minlength=n).astype(np.float32)
    dinv = 1.0 / np.sqrt(np.maximum(deg, 1.0))

    if _HAVE_SCIPY:
        # fold the symmetric normalization into the matrix once:
        # prop(y) = -(D^-1/2 A D^-1/2) @ y
        vals = (-dinv[row] * dinv[col]).astype(np.float32)
        A = sp.csr_matrix((vals, (row, col)), shape=(n, n))
        A.sum_duplicates()
        data2 = (2.0 * A.data).astype(np.float32)

        def prop(y):
            return A @ y

        def prop2_minus(t_cur, t_prev, buf):
            # buf <- 2*A@t_cur - t_prev, accumulated in one SpMM pass
            np.negative(t_prev, out=buf)
            _sparsetools.csr_matvecs(n, n, HID, A.indptr, A.indices,
                                     data2, t_cur.ravel(), buf.ravel())
            return buf
    else:
        order = np.argsort(row, kind="stable")
        rs, cs = row[order], col[order]
        w = (-dinv[rs] * dinv[cs]).astype(np.float32)

        def prop(y):
            s = np.zeros_like(y)
            np.add.at(s, rs, w[:, None] * y[cs])
            return s

    def spectral_layer(h, W, theta, b):
        y = h @ W
        coeff = theta.mean(axis=0)
        t_prev, t_cur = y, prop(y)
        out = coeff[0] * t_prev + coeff[1] * t_cur
        scratch = np.empty_like(y)
        for k in range(2, K):
            if _HAVE_SCIPY:
                buf = np.empty_like(y) if k == 2 else t_prev
                t_next = prop2_minus(t_cur, t_prev, buf)
            else:
                t_next = prop(t_cur)
                np.multiply(t_next, 2.0, out=t_next)
                np.subtract(t_next, t_prev, out=t_next)
            t_prev = t_cur
            t_cur = t_next
            np.multiply(t_next, coeff[k], out=scratch)
            out += scratch
        out += b
        return out

    h = np.maximum(spectral_layer(x, W1, theta1, b1), 0.0)
    for i in range(NUM_LAYERS - 1):
        h = np.maximum(spectral_layer(h, Ws[i], thetas[i], bs[i]), 0.0)

    sums = np.zeros((NUM_GRAPHS, HID), np.float32)
    np.add.at(sums, batch, h)
    cnt = np.bincount(batch, minlength=NUM_GRAPHS).astype(np.float32)
    pooled = sums / np.maximum(cnt, 1.0)[:, None]

    g = np.maximum(pooled @ lin1_w + lin1_b, 0.0)
    logits = g @ lin2_w + lin2_b
    m = logits.max(axis=1, keepdims=True)
    out = logits - m - np.log(np.exp(logits - m).sum(axis=1))[:, None]
    return out.astype(np.float32)



# revision 2
# speedup vs baseline: 3.1623x; 3.1623x over previous
"""ChebNet GNN kernel for nn_Decimation_25142738551433 — Trainium2 SPMD.

Strategy: node-sharded Chebyshev propagation on 8 NeuronCores.
  - CPU: y1 = x @ W1 (BLAS), symmetric-norm edge values, edges counting-sorted
    into a fixed (segment, window, cell) grid -> fp16/int16/u8 arrays.
  - Device (per core, one NEFF for all 3 layers): y state replicated in HBM,
    AllGather per propagation; per 128-edge chunk: dma_gather rows, build
    one-hot scatter matrix on DVE ((iota==row_rel)*val), TensorE matmul into
    PSUM cells, Chebyshev recursion as bulk DVE ops; graph-pooling partial
    sums via one-hot matmul.
  - CPU epilogue: sum 8 pool partials, mean, tiny MLP head, log_softmax.

The Bass program is built/compiled and warmed up at import time; kernel()
only packs inputs, runs the cached jitted SPMD executable, and applies the
epilogue.  Any failure falls back to a scipy CPU implementation.
"""
from dataclasses import dataclass
from contextlib import ExitStack

import numpy as np

N = 100000
E = 1600000
F_IN = 128
HID = 64
K = 14
NUM_LAYERS = 3
NUM_GRAPHS = 128
NUM_CLASSES = 10


@dataclass
class Geom:
    n_cores: int = 8
    hid: int = 64
    kcheb: int = 14
    n_layers: int = 3
    n_graphs: int = 128
    win: int = 98
    segs: int = 4
    cell: int = 5
    bpc: int = 70

    @property
    def rows_core(self):
        return 128 * self.win

    @property
    def nodes_pad(self):
        return self.rows_core * self.n_cores

    @property
    def seg_rows(self):
        return self.nodes_pad // self.segs

    @property
    def chunks_seg(self):
        return self.win * self.cell

    @property
    def n_batches(self):
        return self.chunks_seg // self.bpc

    @property
    def cells_batch(self):
        return self.bpc // self.cell


def _input_specs(g: Geom):
    return {
        "y1h": ([g.rows_core, g.hid], np.float16),
        "colidx": ([16, g.segs, g.n_batches, g.bpc * 8], np.int16),
        "relv": ([128, g.segs, g.n_batches, g.bpc], np.uint8),
        "valv": ([128, g.segs, g.n_batches, g.bpc], np.float16),
        "batchrel": ([128, g.win], np.float16),
        "wts": ([g.hid, (g.n_layers - 1) * g.hid], np.float32),
        "biasc": ([128, g.n_layers * g.hid], np.float32),
        "coefc": ([128, g.n_layers * 16], np.float32),
    }


def _build_gnn(ctx, tc, outs, ins, g: Geom):
    import concourse.bass as bass
    import concourse.mybir as mybir
    from concourse.masks import make_identity
    F32 = mybir.dt.float32
    F16 = mybir.dt.float16
    U8 = mybir.dt.uint8
    I16 = mybir.dt.int16
    AF = mybir.AluOpType

    nc = tc.nc
    H = g.hid
    y1h, colidx, relv, valv, batchrel = (
        ins["y1h"], ins["colidx"], ins["relv"], ins["valv"], ins["batchrel"])
    wts_d, bias_d, coef_d = ins["wts"], ins["biasc"], ins["coefc"]
    pool_out = outs["pool"]

    sb = ctx.enter_context(tc.tile_pool(name="sb", bufs=1))
    spool = ctx.enter_context(tc.tile_pool(name="spool", bufs=4))
    zpool = ctx.enter_context(tc.tile_pool(name="zpool", bufs=1))
    ltp = ctx.enter_context(tc.tile_pool(name="ltp", bufs=2))
    cellps = ctx.enter_context(tc.tile_pool(name="cellps", bufs=4, space="PSUM"))
    t1ps = ctx.enter_context(tc.tile_pool(name="t1ps", bufs=2, space="PSUM"))
    t2ps = ctx.enter_context(tc.tile_pool(name="t2ps", bufs=1, space="PSUM"))
    poolps = ctx.enter_context(tc.tile_pool(name="poolps", bufs=1, space="PSUM"))
    dram = ctx.enter_context(tc.tile_pool(name="dram", bufs=1, space="DRAM"))

    iota = sb.tile([128, 128], F32)
    ident = sb.tile([128, 128], F32)
    colidx_sb = sb.tile([128, g.segs, g.n_batches, g.bpc * 8], I16)
    rel8 = sb.tile([128, g.segs, g.n_batches, g.bpc], U8)
    rel32 = sb.tile([128, g.segs, g.n_batches, g.bpc], F32)
    val16 = sb.tile([128, g.segs, g.n_batches, g.bpc], F16)
    val32 = sb.tile([128, g.segs, g.n_batches, g.bpc], F32)
    brel = sb.tile([128, g.win], F16)
    brel32 = sb.tile([128, g.win], F32)
    wts = sb.tile([H, (g.n_layers - 1) * H], F32)
    biasc = sb.tile([128, g.n_layers * H], F32)
    coefc = sb.tile([128, g.n_layers * 16], F32)
    st0 = sb.tile([128, g.win, H], F32, tag="st0")
    st1 = sb.tile([128, g.win, H], F32, tag="st1")
    st2 = sb.tile([128, g.win, H], F32, tag="st2")
    acc = sb.tile([128, g.win, H], F32, tag="acc")

    nc.gpsimd.iota(iota[:], pattern=[[1, 128]], base=0, channel_multiplier=0,
                   allow_small_or_imprecise_dtypes=True)
    make_identity(nc, ident[:])
    for j in range(8):
        nc.sync.dma_start(colidx_sb[16 * j:16 * (j + 1)], colidx[:])
    nc.sync.dma_start(rel8[:], relv[:])
    nc.sync.dma_start(val16[:], valv[:])
    nc.sync.dma_start(brel[:], batchrel[:])
    nc.sync.dma_start(wts[:], wts_d[:])
    nc.sync.dma_start(biasc[:], bias_d[:])
    nc.sync.dma_start(coefc[:], coef_d[:])
    nc.vector.tensor_copy(rel32[:], rel8[:])
    nc.vector.tensor_copy(val32[:], val16[:])
    nc.vector.tensor_copy(brel32[:], brel[:])

    nc.gpsimd.dma_start(
        out=st0[:], in_=y1h.rearrange("(w p) f -> p w f", p=128))

    state = {"prev": st2, "cur": st0, "scat": st1}

    def shard_to_yfull(t):
        ag_in = dram.tile([g.rows_core, H], F32, tag="ag_in")
        y_full = dram.tile([g.nodes_pad, H], F32, addr_space="Shared",
                           tag="y_full")
        nc.sync.dma_start(
            out=ag_in[:].rearrange("(w p) f -> p w f", p=128), in_=t[:])
        nc.gpsimd.collective_compute(
            "AllGather", AF.bypass,
            replica_groups=[list(range(g.n_cores))],
            ins=[ag_in.opt()], outs=[y_full.opt()])
        return y_full

    def coef_ap(layer, k):
        return coefc[:, layer * 16 + k: layer * 16 + k + 1]

    def bias_bc(layer):
        a = biasc[:, layer * H:(layer + 1) * H]
        return bass.AP(a.tensor, a.offset,
                       [list(a.ap[0]), [0, g.win], list(a.ap[1])])

    def prop(layer, k, y_full):
        t_prev, t_cur, t_scat = state["prev"], state["cur"], state["scat"]
        scale = 1.0 if k == 1 else 2.0
        if k == 1:
            nc.vector.memset(t_scat[:], 0.0)
        else:
            nc.vector.tensor_scalar_mul(t_scat[:], t_prev[:], -1.0)
        with tc.For_i(0, g.segs) as s:
            with tc.For_i(0, g.n_batches) as b:
                zt = zpool.tile([128, g.bpc, H], F32, tag="zt")
                nc.gpsimd.dma_gather(
                    out_ap=zt[:],
                    in_ap=y_full[bass.ds(s * g.seg_rows, g.seg_rows), :],
                    idxs_ap=colidx_sb[:, bass.ds(s, 1), bass.ds(b, 1), :],
                    num_idxs=g.bpc * 128,
                    num_idxs_reg=g.bpc * 128,
                    elem_size=H,
                    single_packet=False,
                )
                for cc in range(g.cells_batch):
                    ps = cellps.tile([128, H], F32, tag="cellps")
                    for j in range(g.cell):
                        c = cc * g.cell + j
                        st = spool.tile([128, 128], F32, tag="sm")
                        nc.vector.tensor_scalar(
                            out=st[:], in0=iota[:],
                            scalar1=rel32[:, bass.ds(s, 1), bass.ds(b, 1),
                                          bass.ds(c, 1)],
                            scalar2=val32[:, bass.ds(s, 1), bass.ds(b, 1),
                                          bass.ds(c, 1)],
                            op0=AF.is_equal, op1=AF.mult)
                        nc.tensor.matmul(
                            ps[:], lhsT=st[:], rhs=zt[:, c, :],
                            start=(j == 0), stop=(j == g.cell - 1))
                    tgt = t_scat[:, bass.ds(b * g.cells_batch + cc, 1), :]
                    nc.vector.scalar_tensor_tensor(
                        out=tgt, in0=ps[:], scalar=scale, in1=tgt,
                        op0=AF.mult, op1=AF.add)
        nc.vector.scalar_tensor_tensor(
            out=acc[:], in0=t_scat[:], scalar=coef_ap(layer, k), in1=acc[:],
            op0=AF.mult, op1=AF.add)
        state["prev"], state["cur"], state["scat"] = t_cur, t_scat, t_prev

    for layer in range(g.n_layers):
        if layer > 0:
            nc.vector.tensor_tensor(
                out=acc[:], in0=acc[:], in1=bias_bc(layer - 1), op=AF.add)
            nc.vector.tensor_scalar_max(acc[:], acc[:], 0.0)
            t_new = state["scat"]
            for w in range(g.win):
                hT_ps = t1ps.tile([H, 128], F32, tag="t1ps")
                nc.tensor.transpose(hT_ps[:], acc[:, w, :], ident[:])
                hT = ltp.tile([H, 128], F32, tag="hT")
                nc.vector.tensor_copy(hT[:], hT_ps[:])
                yT_ps = t1ps.tile([H, 128], F32, tag="t1ps")
                nc.tensor.matmul(
                    yT_ps[:], lhsT=wts[:, (layer - 1) * H:layer * H],
                    rhs=hT[:], start=True, stop=True)
                yT = ltp.tile([H, 128], F32, tag="hT")
                nc.vector.tensor_copy(yT[:], yT_ps[:])
                y_ps = t2ps.tile([128, H], F32, tag="t2ps")
                nc.tensor.transpose(y_ps[:], yT[:], ident[:H, :H])
                nc.vector.tensor_copy(t_new[:, w, :], y_ps[:])
            state["scat"] = state["cur"]
            state["cur"] = t_new
        y_full = shard_to_yfull(state["cur"])
        nc.vector.tensor_scalar(
            out=acc[:], in0=state["cur"][:], scalar1=coef_ap(layer, 0),
            scalar2=None, op0=AF.mult)
        for k in range(1, g.kcheb):
            prop(layer, k, y_full)
            if k < g.kcheb - 1:
                y_full = shard_to_yfull(state["cur"])

    nc.vector.tensor_tensor(
        out=acc[:], in0=acc[:], in1=bias_bc(g.n_layers - 1), op=AF.add)
    nc.vector.tensor_scalar_max(acc[:], acc[:], 0.0)

    pool_ps = poolps.tile([128, H], F32)
    for w in range(g.win):
        pt = spool.tile([128, 128], F32, tag="pt")
        nc.vector.tensor_scalar(
            out=pt[:], in0=iota[:], scalar1=brel32[:, w:w + 1], scalar2=None,
            op0=AF.is_equal)
        nc.tensor.matmul(pool_ps[:], lhsT=pt[:], rhs=acc[:, w, :],
                         start=(w == 0), stop=(w == g.win - 1))
    pool_sb = sb.tile([128, H], F32)
    nc.vector.tensor_copy(pool_sb[:], pool_ps[:])
    nc.sync.dma_start(pool_out[:], pool_sb[:])


class _SpmdRunner:
    def __init__(self, nc, n_cores):
        import jax
        from jax.sharding import Mesh, PartitionSpec
        from jax.experimental.shard_map import shard_map
        from concourse.bass2jax import (
            _bass_exec_p, install_neuronx_cc_hook, partition_id_tensor)
        import concourse.mybir as mybir

        install_neuronx_cc_hook()
        self.n_cores = n_cores
        part_name = (nc.partition_id_tensor.name
                     if nc.partition_id_tensor is not None else None)
        in_names, out_names, out_avals, zero_outs = [], [], [], []
        for alloc in nc.m.functions[0].allocations:
            if not isinstance(alloc, mybir.MemoryLocationSet):
                continue
            name = alloc.memorylocations[0].name
            if alloc.kind == "ExternalInput":
                if name != part_name:
                    in_names.append(name)
            elif alloc.kind == "ExternalOutput":
                aval = jax.core.ShapedArray(
                    tuple(alloc.tensor_shape), mybir.dt.np(alloc.dtype))
                out_names.append(name)
                out_avals.append(aval)
                zero_outs.append(np.zeros(aval.shape, aval.dtype))
        self.n_params = len(in_names)
        self.in_names = list(in_names)
        self.out_names = list(out_names)
        self.out_avals = out_avals
        self.zero_outs = zero_outs
        all_in_names = in_names + out_names
        if part_name is not None:
            all_in_names = all_in_names + [part_name]

        def _body(*args):
            operands = list(args)
            if part_name is not None:
                operands.append(partition_id_tensor())
            return tuple(_bass_exec_p.bind(
                *operands,
                out_avals=tuple(out_avals),
                in_names=tuple(all_in_names),
                out_names=tuple(out_names),
                lowering_input_output_aliases=(),
                sim_require_finite=True,
                sim_require_nnan=True,
                nc=nc,
            ))

        devices = jax.devices()[:n_cores]
        self.mesh = Mesh(np.asarray(devices), ("core",))
        n_outs = len(out_names)
        donate = tuple(range(self.n_params, self.n_params + n_outs))
        self.fn = jax.jit(
            shard_map(_body, mesh=self.mesh,
                      in_specs=(PartitionSpec("core"),) * (self.n_params + n_outs),
                      out_specs=(PartitionSpec("core"),) * n_outs,
                      check_rep=False),
            donate_argnums=donate, keep_unused=True)

    def run(self, concat_by_name):
        concat_in = [concat_by_name[n] for n in self.in_names]
        zeros = [np.zeros((self.n_cores * z.shape[0], *z.shape[1:]), z.dtype)
                 for z in self.zero_outs]
        out_arrs = self.fn(*concat_in, *zeros)
        return {n: np.asarray(out_arrs[i]).reshape(
                    self.n_cores, *self.out_avals[i].shape)
                for i, n in enumerate(self.out_names)}


_GEOM = Geom()
_RUNNER = None
_IMPORT_ERR = None


def _init():
    global _RUNNER, _IMPORT_ERR
    try:
        import concourse.bacc as bacc
        import concourse.mybir as mybir
        import concourse.tile as tile
        g = _GEOM
        nc = bacc.Bacc("TRN2", target_bir_lowering=False, debug=False,
                       num_devices=g.n_cores)
        specs = _input_specs(g)
        ins = {name: nc.dram_tensor(name, shape,
                                    mybir.dt.from_np(np.dtype(dt)),
                                    kind="ExternalInput").ap()
               for name, (shape, dt) in specs.items()}
        outs = {"pool": nc.dram_tensor("pool", [128, g.hid], mybir.dt.float32,
                                       kind="ExternalOutput").ap()}
        with tile.TileContext(nc) as tc:
            with ExitStack() as ctx:
                _build_gnn(ctx, tc, outs, ins, g)
        nc.compile()
        runner = _SpmdRunner(nc, g.n_cores)
        # warm-up with the same arg types as the real call (np arrays):
        # triggers trace + NEFF compile + device load.
        dummy = {name: np.zeros((g.n_cores * s[0], *s[1:]), np.dtype(dt))
                 for name, (s, dt) in specs.items()}
        runner.run(dummy)
        _RUNNER = runner
    except Exception as e:  # fall back to CPU path at call time
        _IMPORT_ERR = e


def _pack_concat(g: Geom, x, edge_index, batch, W1, theta1, b1, Ws, thetas,
                 bs):
    """Concat-layout (axis0 = core-major) input arrays, or None if the input
    does not fit the fixed grid."""
    row = np.ascontiguousarray(edge_index[0]).astype(np.int32)
    col = np.ascontiguousarray(edge_index[1]).astype(np.int32)
    n, e = x.shape[0], row.shape[0]

    deg = np.bincount(row, minlength=n).astype(np.float32)
    dinv = 1.0 / np.sqrt(np.maximum(deg, 1.0))
    val = -(dinv[row] * dinv[col])

    n_win_g = g.win * g.n_cores
    if (n > g.nodes_pad) or (row.max(initial=0) >> 7) >= n_win_g:
        return None
    key = ((row >> 7) * np.int32(g.segs) + col // np.int32(g.seg_rows))
    counts = np.bincount(key, minlength=n_win_g * g.segs)
    if counts.max() > g.cell * 128:
        return None
    order = np.argsort(key.astype(np.uint16), kind="stable")
    k_sorted = key[order]
    starts = np.zeros(n_win_g * g.segs, np.int32)
    np.cumsum(counts[:-1], dtype=np.int32, out=starts[1:])
    pos = np.arange(e, dtype=np.int32) - starts[k_sorted]

    wg = k_sorted // g.segs
    sg = k_sorted % g.segs
    core = wg // g.win
    wl = wg % g.win
    chunk_in_seg = wl * np.int32(g.cell) + (pos >> 7)
    slot = ((sg * np.int32(g.chunks_seg) + chunk_in_seg) << 7) + (pos & 127)
    flat = core * np.int32(g.segs * g.chunks_seg * 128) + slot

    tot = g.n_cores * g.segs * g.chunks_seg * 128
    col16 = np.zeros(tot, np.int16)
    rel8 = np.zeros(tot, np.uint8)
    val16 = np.zeros(tot, np.float16)
    col16[flat] = (col[order] - sg * np.int32(g.seg_rows)).astype(np.int16)
    rel8[flat] = (row[order] & 127).astype(np.uint8)
    val16[flat] = val[order].astype(np.float16)

    nb, bpc = g.n_batches, g.bpc
    i_idx = np.arange(bpc * 128)
    colidx = np.zeros((g.n_cores, g.segs, nb, 16, bpc * 8), np.int16)
    colidx[:, :, :, i_idx % 16, i_idx // 16] = col16.reshape(
        g.n_cores, g.segs, nb, bpc * 128)
    colidx = np.ascontiguousarray(colidx.transpose(0, 3, 1, 2, 4)).reshape(
        g.n_cores * 16, g.segs, nb, bpc * 8)
    rel8 = np.ascontiguousarray(np.moveaxis(
        rel8.reshape(g.n_cores, g.segs, nb, bpc, 128), -1, 1)).reshape(
        g.n_cores * 128, g.segs, nb, bpc)
    val16 = np.ascontiguousarray(np.moveaxis(
        val16.reshape(g.n_cores, g.segs, nb, bpc, 128), -1, 1)).reshape(
        g.n_cores * 128, g.segs, nb, bpc)

    bat = np.full(g.nodes_pad, -1.0, np.float32)
    bat[:n] = batch.astype(np.float32)
    brel = np.ascontiguousarray(np.swapaxes(
        bat.reshape(g.n_cores, g.win, 128), 1, 2)).astype(np.float16).reshape(
        g.n_cores * 128, g.win)

    y1 = x.astype(np.float32) @ W1.astype(np.float32)
    y1p = np.zeros((g.nodes_pad, g.hid), np.float16)
    y1p[:n] = y1.astype(np.float16)

    H = g.hid
    wts = np.ascontiguousarray(
        np.moveaxis(Ws.astype(np.float32), 0, 1)).reshape(H, -1)
    wts = np.tile(wts, (g.n_cores, 1))
    biasc = np.concatenate(
        [np.asarray(b1, np.float32).reshape(1, H)] +
        [np.asarray(bs[i], np.float32).reshape(1, H)
         for i in range(g.n_layers - 1)], axis=1)
    biasc = np.tile(np.broadcast_to(biasc, (128, g.n_layers * H)),
                    (g.n_cores, 1))
    coef = np.zeros((g.n_layers, 16), np.float32)
    coef[0, :g.kcheb] = np.asarray(theta1, np.float32).mean(axis=0)
    for i in range(g.n_layers - 1):
        coef[i + 1, :g.kcheb] = np.asarray(thetas[i], np.float32).mean(axis=0)
    coefc = np.tile(np.broadcast_to(coef.reshape(1, -1),
                                    (128, g.n_layers * 16)), (g.n_cores, 1))

    return {"y1h": y1p, "colidx": colidx, "relv": rel8, "valv": val16,
            "batchrel": brel, "wts": wts, "biasc": biasc, "coefc": coefc}


def _epilogue(g: Geom, pool_parts, batch, lin1_w, lin1_b, lin2_w, lin2_b):
    sums = pool_parts.sum(axis=0)[:g.n_graphs]
    cnt = np.bincount(batch.astype(np.int64),
                      minlength=g.n_graphs).astype(np.float32)
    pooled = sums / np.maximum(cnt, 1.0)[:, None]
    gout = np.maximum(pooled @ lin1_w + lin1_b, 0.0)
    logits = gout @ lin2_w + lin2_b
    m = logits.max(axis=1, keepdims=True)
    out = logits - m - np.log(np.exp(logits - m).sum(axis=1))[:, None]
    return out.astype(np.float32)


def _kernel_cpu(x, edge_index, batch, W1, theta1, b1, Ws, thetas, bs,
                lin1_w, lin1_b, lin2_w, lin2_b):
    """scipy fallback (the previous baseline)."""
    import scipy.sparse as sp
    x = np.asarray(x, np.float32)
    row = np.asarray(edge_index[0]).astype(np.int64)
    col = np.asarray(edge_index[1]).astype(np.int64)
    n = x.shape[0]
    deg = np.bincount(row, minlength=n).astype(np.float32)
    dinv = 1.0 / np.sqrt(np.maximum(deg, 1.0))
    vals = (-dinv[row] * dinv[col]).astype(np.float32)
    A = sp.csr_matrix((vals, (row, col)), shape=(n, n))

    def spectral_layer(h, W, theta, b):
        y = h @ np.asarray(W, np.float32)
        coeff = np.asarray(theta, np.float32).mean(axis=0)
        t_prev, t_cur = y, A @ y
        out = coeff[0] * t_prev + coeff[1] * t_cur
        for k in range(2, K):
            t_next = 2.0 * (A @ t_cur) - t_prev
            out = out + coeff[k] * t_next
            t_prev, t_cur = t_cur, t_next
        return out + np.asarray(b, np.float32)

    h = np.maximum(spectral_layer(x, W1, theta1, b1), 0.0)
    for i in range(NUM_LAYERS - 1):
        h = np.maximum(spectral_layer(h, Ws[i], thetas[i], bs[i]), 0.0)
    sums = np.zeros((NUM_GRAPHS, HID), np.float32)
    np.add.at(sums, np.asarray(batch, np.int64), h)
    cnt = np.bincount(np.asarray(batch, np.int64),
                      minlength=NUM_GRAPHS).astype(np.float32)
    pooled = sums / np.maximum(cnt, 1.0)[:, None]
    g = np.maximum(pooled @ np.asarray(lin1_w, np.float32) + lin1_b, 0.0)
    logits = g @ np.asarray(lin2_w, np.float32) + lin2_b
    m = logits.max(axis=1, keepdims=True)
    out = logits - m - np.log(np.exp(logits - m).sum(axis=1))[:, None]
    return out.astype(np.float32)


def kernel(x, edge_index, batch, W1, theta1, b1, Ws, thetas, bs,
           lin1_w, lin1_b, lin2_w, lin2_b):
    try:
        if _RUNNER is None:
            raise RuntimeError(f"no trn2 runner: {_IMPORT_ERR}")
        g = _GEOM
        x = np.asarray(x)
        if x.shape != (N, F_IN):
            raise RuntimeError("unexpected shape")
        packed = _pack_concat(g, x, np.asarray(edge_index), np.asarray(batch),
                              np.asarray(W1), np.asarray(theta1),
                              np.asarray(b1), np.asarray(Ws),
                              np.asarray(thetas), np.asarray(bs))
        if packed is None:
            raise RuntimeError("grid capacity exceeded")
        res = _RUNNER.run(packed)
        pool_parts = res["pool"]
        if not np.isfinite(pool_parts).all():
            raise RuntimeError("non-finite device result")
        return _epilogue(g, pool_parts, np.asarray(batch),
                         np.asarray(lin1_w, np.float32),
                         np.asarray(lin1_b, np.float32),
                         np.asarray(lin2_w, np.float32),
                         np.asarray(lin2_b, np.float32))
    except Exception:
        return _kernel_cpu(x, edge_index, batch, W1, theta1, b1, Ws, thetas,
                           bs, lin1_w, lin1_b, lin2_w, lin2_b)


_init()


# revision 3
# speedup vs baseline: 3.7433x; 1.1837x over previous
"""ChebNet GNN kernel for nn_Decimation_25142738551433 — Trainium2 SPMD.

Strategy: node-sharded Chebyshev propagation on 8 NeuronCores.
  - CPU: y1 = x @ W1 (BLAS), symmetric-norm edge values, edges counting-sorted
    into a fixed (segment, window, cell) grid -> fp16/int16/u8 arrays.
  - Device (per core, one NEFF for all 3 layers): y state replicated in HBM,
    AllGather per propagation; per 128-edge chunk: dma_gather rows, build
    one-hot scatter matrix on DVE ((iota==row_rel)*val), TensorE matmul into
    PSUM cells, Chebyshev recursion as bulk DVE ops; graph-pooling partial
    sums via one-hot matmul.
  - CPU epilogue: sum 8 pool partials, mean, tiny MLP head, log_softmax.

The Bass program is built/compiled and warmed up at import time; kernel()
only packs inputs, runs the cached jitted SPMD executable, and applies the
epilogue.  Any failure falls back to a scipy CPU implementation.
"""
from dataclasses import dataclass
from contextlib import ExitStack

import numpy as np

N = 100000
E = 1600000
F_IN = 128
HID = 64
K = 14
NUM_LAYERS = 3
NUM_GRAPHS = 128
NUM_CLASSES = 10


@dataclass
class Geom:
    n_cores: int = 8
    hid: int = 64
    kcheb: int = 14
    n_layers: int = 3
    n_graphs: int = 128
    win: int = 98
    segs: int = 4
    cell: int = 5
    bpc: int = 70

    @property
    def rows_core(self):
        return 128 * self.win

    @property
    def nodes_pad(self):
        return self.rows_core * self.n_cores

    @property
    def seg_rows(self):
        return self.nodes_pad // self.segs

    @property
    def chunks_seg(self):
        return self.win * self.cell

    @property
    def n_batches(self):
        return self.chunks_seg // self.bpc

    @property
    def cells_batch(self):
        return self.bpc // self.cell


def _input_specs(g: Geom):
    return {
        "y1h": ([g.rows_core, g.hid], np.float16),
        "colidx": ([16, g.segs, g.n_batches, g.bpc * 8], np.int16),
        "relv": ([128, g.segs, g.n_batches, g.bpc], np.uint8),
        "valv": ([128, g.segs, g.n_batches, g.bpc], np.float16),
        "batchrel": ([128, g.win], np.float16),
        "wts": ([g.hid, (g.n_layers - 1) * g.hid], np.float32),
        "biasc": ([128, g.n_layers * g.hid], np.float32),
        "coefc": ([128, g.n_layers * 16], np.float32),
    }


def _build_gnn(ctx, tc, outs, ins, g: Geom):
    import concourse.bass as bass
    import concourse.mybir as mybir
    from concourse.masks import make_identity
    F32 = mybir.dt.float32
    F16 = mybir.dt.float16
    U8 = mybir.dt.uint8
    I16 = mybir.dt.int16
    AF = mybir.AluOpType

    nc = tc.nc
    H = g.hid
    y1h, colidx, relv, valv, batchrel = (
        ins["y1h"], ins["colidx"], ins["relv"], ins["valv"], ins["batchrel"])
    wts_d, bias_d, coef_d = ins["wts"], ins["biasc"], ins["coefc"]
    pool_out = outs["pool"]

    sb = ctx.enter_context(tc.tile_pool(name="sb", bufs=1))
    spool = ctx.enter_context(tc.tile_pool(name="spool", bufs=4))
    zpool = ctx.enter_context(tc.tile_pool(name="zpool", bufs=1))
    ltp = ctx.enter_context(tc.tile_pool(name="ltp", bufs=2))
    cellps = ctx.enter_context(tc.tile_pool(name="cellps", bufs=4, space="PSUM"))
    t1ps = ctx.enter_context(tc.tile_pool(name="t1ps", bufs=2, space="PSUM"))
    t2ps = ctx.enter_context(tc.tile_pool(name="t2ps", bufs=1, space="PSUM"))
    poolps = ctx.enter_context(tc.tile_pool(name="poolps", bufs=1, space="PSUM"))
    dram = ctx.enter_context(tc.tile_pool(name="dram", bufs=1, space="DRAM"))

    iota = sb.tile([128, 128], F32)
    ident = sb.tile([128, 128], F32)
    colidx_sb = sb.tile([128, g.segs, g.n_batches, g.bpc * 8], I16)
    rel8 = sb.tile([128, g.segs, g.n_batches, g.bpc], U8)
    rel32 = sb.tile([128, g.segs, g.n_batches, g.bpc], F32)
    val16 = sb.tile([128, g.segs, g.n_batches, g.bpc], F16)
    val32 = sb.tile([128, g.segs, g.n_batches, g.bpc], F32)
    brel = sb.tile([128, g.win], F16)
    brel32 = sb.tile([128, g.win], F32)
    wts = sb.tile([H, (g.n_layers - 1) * H], F32)
    biasc = sb.tile([128, g.n_layers * H], F32)
    coefc = sb.tile([128, g.n_layers * 16], F32)
    st0 = sb.tile([128, g.win, H], F32, tag="st0")
    st1 = sb.tile([128, g.win, H], F32, tag="st1")
    st2 = sb.tile([128, g.win, H], F32, tag="st2")
    acc = sb.tile([128, g.win, H], F32, tag="acc")

    nc.gpsimd.iota(iota[:], pattern=[[1, 128]], base=0, channel_multiplier=0,
                   allow_small_or_imprecise_dtypes=True)
    make_identity(nc, ident[:])
    for j in range(8):
        nc.sync.dma_start(colidx_sb[16 * j:16 * (j + 1)], colidx[:])
    nc.sync.dma_start(rel8[:], relv[:])
    nc.sync.dma_start(val16[:], valv[:])
    nc.sync.dma_start(brel[:], batchrel[:])
    nc.sync.dma_start(wts[:], wts_d[:])
    nc.sync.dma_start(biasc[:], bias_d[:])
    nc.sync.dma_start(coefc[:], coef_d[:])
    nc.vector.tensor_copy(rel32[:], rel8[:])
    nc.vector.tensor_copy(val32[:], val16[:])
    nc.vector.tensor_copy(brel32[:], brel[:])

    nc.gpsimd.dma_start(
        out=st0[:], in_=y1h.rearrange("(w p) f -> p w f", p=128))

    state = {"prev": st2, "cur": st0, "scat": st1}

    def shard_to_yfull(t):
        ag_in = dram.tile([g.rows_core, H], F32, tag="ag_in")
        y_full = dram.tile([g.nodes_pad, H], F32, addr_space="Shared",
                           tag="y_full")
        nc.sync.dma_start(
            out=ag_in[:].rearrange("(w p) f -> p w f", p=128), in_=t[:])
        nc.gpsimd.collective_compute(
            "AllGather", AF.bypass,
            replica_groups=[list(range(g.n_cores))],
            ins=[ag_in.opt()], outs=[y_full.opt()])
        return y_full

    def coef_ap(layer, k):
        return coefc[:, layer * 16 + k: layer * 16 + k + 1]

    def bias_bc(layer):
        a = biasc[:, layer * H:(layer + 1) * H]
        return bass.AP(a.tensor, a.offset,
                       [list(a.ap[0]), [0, g.win], list(a.ap[1])])

    def prop(layer, k, y_full):
        t_prev, t_cur, t_scat = state["prev"], state["cur"], state["scat"]
        scale = 1.0 if k == 1 else 2.0
        if k == 1:
            nc.vector.memset(t_scat[:], 0.0)
        else:
            nc.vector.tensor_scalar_mul(t_scat[:], t_prev[:], -1.0)
        with tc.For_i(0, g.segs) as s:
            with tc.For_i(0, g.n_batches) as b:
                zt = zpool.tile([128, g.bpc, H], F32, tag="zt")
                nc.gpsimd.dma_gather(
                    out_ap=zt[:],
                    in_ap=y_full[bass.ds(s * g.seg_rows, g.seg_rows), :],
                    idxs_ap=colidx_sb[:, bass.ds(s, 1), bass.ds(b, 1), :],
                    num_idxs=g.bpc * 128,
                    num_idxs_reg=g.bpc * 128,
                    elem_size=H,
                    single_packet=False,
                )
                for cc in range(g.cells_batch):
                    ps = cellps.tile([128, H], F32, tag="cellps")
                    for j in range(g.cell):
                        c = cc * g.cell + j
                        st = spool.tile([128, 128], F32, tag="sm")
                        nc.vector.tensor_scalar(
                            out=st[:], in0=iota[:],
                            scalar1=rel32[:, bass.ds(s, 1), bass.ds(b, 1),
                                          bass.ds(c, 1)],
                            scalar2=val32[:, bass.ds(s, 1), bass.ds(b, 1),
                                          bass.ds(c, 1)],
                            op0=AF.is_equal, op1=AF.mult)
                        nc.tensor.matmul(
                            ps[:], lhsT=st[:], rhs=zt[:, c, :],
                            start=(j == 0), stop=(j == g.cell - 1))
                    tgt = t_scat[:, bass.ds(b * g.cells_batch + cc, 1), :]
                    nc.vector.scalar_tensor_tensor(
                        out=tgt, in0=ps[:], scalar=scale, in1=tgt,
                        op0=AF.mult, op1=AF.add)
        nc.vector.scalar_tensor_tensor(
            out=acc[:], in0=t_scat[:], scalar=coef_ap(layer, k), in1=acc[:],
            op0=AF.mult, op1=AF.add)
        state["prev"], state["cur"], state["scat"] = t_cur, t_scat, t_prev

    for layer in range(g.n_layers):
        if layer > 0:
            nc.vector.tensor_tensor(
                out=acc[:], in0=acc[:], in1=bias_bc(layer - 1), op=AF.add)
            nc.vector.tensor_scalar_max(acc[:], acc[:], 0.0)
            t_new = state["scat"]
            for w in range(g.win):
                hT_ps = t1ps.tile([H, 128], F32, tag="t1ps")
                nc.tensor.transpose(hT_ps[:], acc[:, w, :], ident[:])
                hT = ltp.tile([H, 128], F32, tag="hT")
                nc.vector.tensor_copy(hT[:], hT_ps[:])
                yT_ps = t1ps.tile([H, 128], F32, tag="t1ps")
                nc.tensor.matmul(
                    yT_ps[:], lhsT=wts[:, (layer - 1) * H:layer * H],
                    rhs=hT[:], start=True, stop=True)
                yT = ltp.tile([H, 128], F32, tag="hT")
                nc.vector.tensor_copy(yT[:], yT_ps[:])
                y_ps = t2ps.tile([128, H], F32, tag="t2ps")
                nc.tensor.transpose(y_ps[:], yT[:], ident[:H, :H])
                nc.vector.tensor_copy(t_new[:, w, :], y_ps[:])
            state["scat"] = state["cur"]
            state["cur"] = t_new
        y_full = shard_to_yfull(state["cur"])
        nc.vector.tensor_scalar(
            out=acc[:], in0=state["cur"][:], scalar1=coef_ap(layer, 0),
            scalar2=None, op0=AF.mult)
        for k in range(1, g.kcheb):
            prop(layer, k, y_full)
            if k < g.kcheb - 1:
                y_full = shard_to_yfull(state["cur"])

    nc.vector.tensor_tensor(
        out=acc[:], in0=acc[:], in1=bias_bc(g.n_layers - 1), op=AF.add)
    nc.vector.tensor_scalar_max(acc[:], acc[:], 0.0)

    pool_ps = poolps.tile([128, H], F32)
    for w in range(g.win):
        pt = spool.tile([128, 128], F32, tag="pt")
        nc.vector.tensor_scalar(
            out=pt[:], in0=iota[:], scalar1=brel32[:, w:w + 1], scalar2=None,
            op0=AF.is_equal)
        nc.tensor.matmul(pool_ps[:], lhsT=pt[:], rhs=acc[:, w, :],
                         start=(w == 0), stop=(w == g.win - 1))
    pool_sb = sb.tile([128, H], F32)
    nc.vector.tensor_copy(pool_sb[:], pool_ps[:])
    nc.sync.dma_start(pool_out[:], pool_sb[:])


class _SpmdRunner:
    def __init__(self, nc, n_cores):
        import jax
        from jax.sharding import Mesh, PartitionSpec
        from jax.experimental.shard_map import shard_map
        from concourse.bass2jax import (
            _bass_exec_p, install_neuronx_cc_hook, partition_id_tensor)
        import concourse.mybir as mybir

        install_neuronx_cc_hook()
        self.n_cores = n_cores
        part_name = (nc.partition_id_tensor.name
                     if nc.partition_id_tensor is not None else None)
        in_names, out_names, out_avals, zero_outs = [], [], [], []
        for alloc in nc.m.functions[0].allocations:
            if not isinstance(alloc, mybir.MemoryLocationSet):
                continue
            name = alloc.memorylocations[0].name
            if alloc.kind == "ExternalInput":
                if name != part_name:
                    in_names.append(name)
            elif alloc.kind == "ExternalOutput":
                aval = jax.core.ShapedArray(
                    tuple(alloc.tensor_shape), mybir.dt.np(alloc.dtype))
                out_names.append(name)
                out_avals.append(aval)
                zero_outs.append(np.zeros(aval.shape, aval.dtype))
        self.n_params = len(in_names)
        self.in_names = list(in_names)
        self.out_names = list(out_names)
        self.out_avals = out_avals
        self.zero_outs = zero_outs
        all_in_names = in_names + out_names
        if part_name is not None:
            all_in_names = all_in_names + [part_name]

        def _body(*args):
            operands = list(args)
            if part_name is not None:
                operands.append(partition_id_tensor())
            return tuple(_bass_exec_p.bind(
                *operands,
                out_avals=tuple(out_avals),
                in_names=tuple(all_in_names),
                out_names=tuple(out_names),
                lowering_input_output_aliases=(),
                sim_require_finite=True,
                sim_require_nnan=True,
                nc=nc,
            ))

        devices = jax.devices()[:n_cores]
        self.mesh = Mesh(np.asarray(devices), ("core",))
        n_outs = len(out_names)
        donate = tuple(range(self.n_params, self.n_params + n_outs))
        self.fn = jax.jit(
            shard_map(_body, mesh=self.mesh,
                      in_specs=(PartitionSpec("core"),) * (self.n_params + n_outs),
                      out_specs=(PartitionSpec("core"),) * n_outs,
                      check_rep=False),
            donate_argnums=donate, keep_unused=True)

    def run(self, concat_by_name):
        concat_in = [concat_by_name[n] for n in self.in_names]
        zeros = [np.zeros((self.n_cores * z.shape[0], *z.shape[1:]), z.dtype)
                 for z in self.zero_outs]
        out_arrs = self.fn(*concat_in, *zeros)
        return {n: np.asarray(out_arrs[i]).reshape(
                    self.n_cores, *self.out_avals[i].shape)
                for i, n in enumerate(self.out_names)}


_GEOM = Geom()
_RUNNER = None
_IMPORT_ERR = None


def _init():
    global _RUNNER, _IMPORT_ERR
    try:
        import concourse.bacc as bacc
        import concourse.mybir as mybir
        import concourse.tile as tile
        g = _GEOM
        nc = bacc.Bacc("TRN2", target_bir_lowering=False, debug=False,
                       num_devices=g.n_cores)
        specs = _input_specs(g)
        ins = {name: nc.dram_tensor(name, shape,
                                    mybir.dt.from_np(np.dtype(dt)),
                                    kind="ExternalInput").ap()
               for name, (shape, dt) in specs.items()}
        outs = {"pool": nc.dram_tensor("pool", [128, g.hid], mybir.dt.float32,
                                       kind="ExternalOutput").ap()}
        with tile.TileContext(nc) as tc:
            with ExitStack() as ctx:
                _build_gnn(ctx, tc, outs, ins, g)
        nc.compile()
        runner = _SpmdRunner(nc, g.n_cores)
        # warm-up with the same arg types as the real call (y1h pre-put
        # as a sharded device array, the rest np): triggers trace + NEFF
        # compile + device load.
        import jax
        from jax.sharding import NamedSharding, PartitionSpec
        sh = NamedSharding(runner.mesh, PartitionSpec("core"))
        runner.sharding = sh
        dummy = {name: np.zeros((g.n_cores * s[0], *s[1:]), np.dtype(dt))
                 for name, (s, dt) in specs.items()}
        dummy["y1h"] = jax.device_put(dummy["y1h"], sh)
        runner.run(dummy)
        _RUNNER = runner
    except Exception as e:  # fall back to CPU path at call time
        _IMPORT_ERR = e


def _pack_concat(g: Geom, x, edge_index, batch, W1, theta1, b1, Ws, thetas,
                 bs):
    """Concat-layout (axis0 = core-major) input arrays, or None if the input
    does not fit the fixed grid."""
    row = np.ascontiguousarray(edge_index[0]).astype(np.int32)
    col = np.ascontiguousarray(edge_index[1]).astype(np.int32)
    n, e = x.shape[0], row.shape[0]

    deg = np.bincount(row, minlength=n).astype(np.float32)
    dinv = 1.0 / np.sqrt(np.maximum(deg, 1.0))
    val = -(dinv[row] * dinv[col])

    n_win_g = g.win * g.n_cores
    if (n > g.nodes_pad) or (row.max(initial=0) >> 7) >= n_win_g:
        return None
    key = ((row >> 7) * np.int32(g.segs) + col // np.int32(g.seg_rows))
    counts = np.bincount(key, minlength=n_win_g * g.segs)
    if counts.max() > g.cell * 128:
        return None
    order = np.argsort(key.astype(np.uint16), kind="stable")
    k_sorted = key[order]
    starts = np.zeros(n_win_g * g.segs, np.int32)
    np.cumsum(counts[:-1], dtype=np.int32, out=starts[1:])
    pos = np.arange(e, dtype=np.int32) - starts[k_sorted]

    wg = k_sorted // g.segs
    sg = k_sorted % g.segs
    core = wg // g.win
    wl = wg % g.win
    chunk_in_seg = wl * np.int32(g.cell) + (pos >> 7)
    slot = ((sg * np.int32(g.chunks_seg) + chunk_in_seg) << 7) + (pos & 127)
    flat = core * np.int32(g.segs * g.chunks_seg * 128) + slot

    tot = g.n_cores * g.segs * g.chunks_seg * 128
    col16 = np.zeros(tot, np.int16)
    rel8 = np.zeros(tot, np.uint8)
    val16 = np.zeros(tot, np.float16)
    col16[flat] = (col[order] - sg * np.int32(g.seg_rows)).astype(np.int16)
    rel8[flat] = (row[order] & 127).astype(np.uint8)
    val16[flat] = val[order].astype(np.float16)

    nb, bpc = g.n_batches, g.bpc
    i_idx = np.arange(bpc * 128)
    colidx = np.zeros((g.n_cores, g.segs, nb, 16, bpc * 8), np.int16)
    colidx[:, :, :, i_idx % 16, i_idx // 16] = col16.reshape(
        g.n_cores, g.segs, nb, bpc * 128)
    colidx = np.ascontiguousarray(colidx.transpose(0, 3, 1, 2, 4)).reshape(
        g.n_cores * 16, g.segs, nb, bpc * 8)
    rel8 = np.ascontiguousarray(np.moveaxis(
        rel8.reshape(g.n_cores, g.segs, nb, bpc, 128), -1, 1)).reshape(
        g.n_cores * 128, g.segs, nb, bpc)
    val16 = np.ascontiguousarray(np.moveaxis(
        val16.reshape(g.n_cores, g.segs, nb, bpc, 128), -1, 1)).reshape(
        g.n_cores * 128, g.segs, nb, bpc)

    bat = np.full(g.nodes_pad, -1.0, np.float32)
    bat[:n] = batch.astype(np.float32)
    brel = np.ascontiguousarray(np.swapaxes(
        bat.reshape(g.n_cores, g.win, 128), 1, 2)).astype(np.float16).reshape(
        g.n_cores * 128, g.win)

    H = g.hid
    wts = np.ascontiguousarray(
        np.moveaxis(Ws.astype(np.float32), 0, 1)).reshape(H, -1)
    wts = np.tile(wts, (g.n_cores, 1))
    biasc = np.concatenate(
        [np.asarray(b1, np.float32).reshape(1, H)] +
        [np.asarray(bs[i], np.float32).reshape(1, H)
         for i in range(g.n_layers - 1)], axis=1)
    biasc = np.tile(np.broadcast_to(biasc, (128, g.n_layers * H)),
                    (g.n_cores, 1))
    coef = np.zeros((g.n_layers, 16), np.float32)
    coef[0, :g.kcheb] = np.asarray(theta1, np.float32).mean(axis=0)
    for i in range(g.n_layers - 1):
        coef[i + 1, :g.kcheb] = np.asarray(thetas[i], np.float32).mean(axis=0)
    coefc = np.tile(np.broadcast_to(coef.reshape(1, -1),
                                    (128, g.n_layers * 16)), (g.n_cores, 1))

    return {"colidx": colidx, "relv": rel8, "valv": val16,
            "batchrel": brel, "wts": wts, "biasc": biasc, "coefc": coefc}


def _pack_y1(g: Geom, x, W1):
    n = x.shape[0]
    y1 = np.asarray(x, np.float32) @ np.asarray(W1, np.float32)
    y1p = np.zeros((g.nodes_pad, g.hid), np.float16)
    y1p[:n] = y1.astype(np.float16)
    return y1p


def _epilogue(g: Geom, pool_parts, batch, lin1_w, lin1_b, lin2_w, lin2_b):
    sums = pool_parts.sum(axis=0)[:g.n_graphs]
    cnt = np.bincount(batch.astype(np.int64),
                      minlength=g.n_graphs).astype(np.float32)
    pooled = sums / np.maximum(cnt, 1.0)[:, None]
    gout = np.maximum(pooled @ lin1_w + lin1_b, 0.0)
    logits = gout @ lin2_w + lin2_b
    m = logits.max(axis=1, keepdims=True)
    out = logits - m - np.log(np.exp(logits - m).sum(axis=1))[:, None]
    return out.astype(np.float32)


def _kernel_cpu(x, edge_index, batch, W1, theta1, b1, Ws, thetas, bs,
                lin1_w, lin1_b, lin2_w, lin2_b):
    """scipy fallback (the previous baseline)."""
    import scipy.sparse as sp
    x = np.asarray(x, np.float32)
    row = np.asarray(edge_index[0]).astype(np.int64)
    col = np.asarray(edge_index[1]).astype(np.int64)
    n = x.shape[0]
    deg = np.bincount(row, minlength=n).astype(np.float32)
    dinv = 1.0 / np.sqrt(np.maximum(deg, 1.0))
    vals = (-dinv[row] * dinv[col]).astype(np.float32)
    A = sp.csr_matrix((vals, (row, col)), shape=(n, n))

    def spectral_layer(h, W, theta, b):
        y = h @ np.asarray(W, np.float32)
        coeff = np.asarray(theta, np.float32).mean(axis=0)
        t_prev, t_cur = y, A @ y
        out = coeff[0] * t_prev + coeff[1] * t_cur
        for k in range(2, K):
            t_next = 2.0 * (A @ t_cur) - t_prev
            out = out + coeff[k] * t_next
            t_prev, t_cur = t_cur, t_next
        return out + np.asarray(b, np.float32)

    h = np.maximum(spectral_layer(x, W1, theta1, b1), 0.0)
    for i in range(NUM_LAYERS - 1):
        h = np.maximum(spectral_layer(h, Ws[i], thetas[i], bs[i]), 0.0)
    sums = np.zeros((NUM_GRAPHS, HID), np.float32)
    np.add.at(sums, np.asarray(batch, np.int64), h)
    cnt = np.bincount(np.asarray(batch, np.int64),
                      minlength=NUM_GRAPHS).astype(np.float32)
    pooled = sums / np.maximum(cnt, 1.0)[:, None]
    g = np.maximum(pooled @ np.asarray(lin1_w, np.float32) + lin1_b, 0.0)
    logits = g @ np.asarray(lin2_w, np.float32) + lin2_b
    m = logits.max(axis=1, keepdims=True)
    out = logits - m - np.log(np.exp(logits - m).sum(axis=1))[:, None]
    return out.astype(np.float32)


def kernel(x, edge_index, batch, W1, theta1, b1, Ws, thetas, bs,
           lin1_w, lin1_b, lin2_w, lin2_b):
    try:
        if _RUNNER is None:
            raise RuntimeError(f"no trn2 runner: {_IMPORT_ERR}")
        g = _GEOM
        x = np.asarray(x)
        if x.shape != (N, F_IN):
            raise RuntimeError("unexpected shape")
        import jax
        y1p = _pack_y1(g, x, W1)
        y1_dev = jax.device_put(y1p, _RUNNER.sharding)  # async upload
        packed = _pack_concat(g, x, np.asarray(edge_index), np.asarray(batch),
                              np.asarray(W1), np.asarray(theta1),
                              np.asarray(b1), np.asarray(Ws),
                              np.asarray(thetas), np.asarray(bs))
        if packed is None:
            raise RuntimeError("grid capacity exceeded")
        packed["y1h"] = y1_dev
        res = _RUNNER.run(packed)
        pool_parts = res["pool"]
        if not np.isfinite(pool_parts).all():
            raise RuntimeError("non-finite device result")
        return _epilogue(g, pool_parts, np.asarray(batch),
                         np.asarray(lin1_w, np.float32),
                         np.asarray(lin1_b, np.float32),
                         np.asarray(lin2_w, np.float32),
                         np.asarray(lin2_b, np.float32))
    except Exception:
        return _kernel_cpu(x, edge_index, batch, W1, theta1, b1, Ws, thetas,
                           bs, lin1_w, lin1_b, lin2_w, lin2_b)


_init()


# revision 4
# speedup vs baseline: 3.9142x; 1.0457x over previous
"""ChebNet GNN kernel for nn_Decimation_25142738551433 — Trainium2 SPMD.

Strategy: node-sharded Chebyshev propagation on 8 NeuronCores.
  - CPU: y1 = x @ W1 (BLAS), symmetric-norm edge values, edges counting-sorted
    into a fixed (segment, window, cell) grid -> fp16/int16/u8 arrays.
  - Device (per core, one NEFF for all 3 layers): y state replicated in HBM,
    AllGather per propagation; per 128-edge chunk: dma_gather rows, build
    one-hot scatter matrix on DVE ((iota==row_rel)*val), TensorE matmul into
    PSUM cells, Chebyshev recursion as bulk DVE ops; graph-pooling partial
    sums via one-hot matmul.
  - CPU epilogue: sum 8 pool partials, mean, tiny MLP head, log_softmax.

The Bass program is built/compiled and warmed up at import time; kernel()
only packs inputs, runs the cached jitted SPMD executable, and applies the
epilogue.  Any failure falls back to a scipy CPU implementation.
"""
from dataclasses import dataclass
from contextlib import ExitStack

import numpy as np

N = 100000
E = 1600000
F_IN = 128
HID = 64
K = 14
NUM_LAYERS = 3
NUM_GRAPHS = 128
NUM_CLASSES = 10


@dataclass
class Geom:
    n_cores: int = 8
    hid: int = 64
    kcheb: int = 14
    n_layers: int = 3
    n_graphs: int = 128
    win: int = 98
    segs: int = 4
    cell: int = 5
    bpc: int = 70

    @property
    def rows_core(self):
        return 128 * self.win

    @property
    def nodes_pad(self):
        return self.rows_core * self.n_cores

    @property
    def seg_rows(self):
        return self.nodes_pad // self.segs

    @property
    def chunks_seg(self):
        return self.win * self.cell

    @property
    def n_batches(self):
        return self.chunks_seg // self.bpc

    @property
    def cells_batch(self):
        return self.bpc // self.cell


def _input_specs(g: Geom):
    return {
        "y1h": ([g.rows_core, g.hid], np.float16),
        "colidx": ([16, g.segs, g.n_batches, g.bpc * 8], np.int16),
        "relv": ([128, g.segs, g.n_batches, g.bpc], np.uint8),
        "valv": ([128, g.segs, g.n_batches, g.bpc], np.float16),
        "batchrel": ([128, g.win], np.float16),
        "wts": ([g.hid, (g.n_layers - 1) * g.hid], np.float32),
        "biasc": ([128, g.n_layers * g.hid], np.float32),
        "coefc": ([128, g.n_layers * 16], np.float32),
    }


def _build_gnn(ctx, tc, outs, ins, g: Geom):
    import concourse.bass as bass
    import concourse.mybir as mybir
    from concourse.masks import make_identity
    F32 = mybir.dt.float32
    F16 = mybir.dt.float16
    U8 = mybir.dt.uint8
    I16 = mybir.dt.int16
    AF = mybir.AluOpType

    nc = tc.nc
    H = g.hid
    y1h, colidx, relv, valv, batchrel = (
        ins["y1h"], ins["colidx"], ins["relv"], ins["valv"], ins["batchrel"])
    wts_d, bias_d, coef_d = ins["wts"], ins["biasc"], ins["coefc"]
    pool_out = outs["pool"]

    sb = ctx.enter_context(tc.tile_pool(name="sb", bufs=1))
    spool = ctx.enter_context(tc.tile_pool(name="spool", bufs=4))
    zpool = ctx.enter_context(tc.tile_pool(name="zpool", bufs=2))
    ltp = ctx.enter_context(tc.tile_pool(name="ltp", bufs=2))
    cellps = ctx.enter_context(tc.tile_pool(name="cellps", bufs=4, space="PSUM"))
    t1ps = ctx.enter_context(tc.tile_pool(name="t1ps", bufs=2, space="PSUM"))
    t2ps = ctx.enter_context(tc.tile_pool(name="t2ps", bufs=1, space="PSUM"))
    poolps = ctx.enter_context(tc.tile_pool(name="poolps", bufs=1, space="PSUM"))
    dram = ctx.enter_context(tc.tile_pool(name="dram", bufs=1, space="DRAM"))

    iota = sb.tile([128, 128], F32)
    ident = sb.tile([128, 128], F32)
    colidx_sb = sb.tile([128, g.segs, g.n_batches, g.bpc * 8], I16)
    rel8 = sb.tile([128, g.segs, g.n_batches, g.bpc], U8)
    rel32 = sb.tile([128, g.segs, g.n_batches, g.bpc], F32)
    val16 = sb.tile([128, g.segs, g.n_batches, g.bpc], F16)
    val32 = sb.tile([128, g.segs, g.n_batches, g.bpc], F32)
    brel = sb.tile([128, g.win], F16)
    brel32 = sb.tile([128, g.win], F32)
    wts = sb.tile([H, (g.n_layers - 1) * H], F32)
    biasc = sb.tile([128, g.n_layers * H], F32)
    coefc = sb.tile([128, g.n_layers * 16], F32)
    st0 = sb.tile([128, g.win, H], F32, tag="st0")
    st1 = sb.tile([128, g.win, H], F32, tag="st1")
    st2 = sb.tile([128, g.win, H], F32, tag="st2")
    acc = sb.tile([128, g.win, H], F32, tag="acc")

    nc.gpsimd.iota(iota[:], pattern=[[1, 128]], base=0, channel_multiplier=0,
                   allow_small_or_imprecise_dtypes=True)
    make_identity(nc, ident[:])
    for j in range(8):
        nc.sync.dma_start(colidx_sb[16 * j:16 * (j + 1)], colidx[:])
    nc.sync.dma_start(rel8[:], relv[:])
    nc.sync.dma_start(val16[:], valv[:])
    nc.sync.dma_start(brel[:], batchrel[:])
    nc.sync.dma_start(wts[:], wts_d[:])
    nc.sync.dma_start(biasc[:], bias_d[:])
    nc.sync.dma_start(coefc[:], coef_d[:])
    nc.vector.tensor_copy(rel32[:], rel8[:])
    nc.vector.tensor_copy(val32[:], val16[:])
    nc.vector.tensor_copy(brel32[:], brel[:])

    nc.gpsimd.dma_start(
        out=st0[:], in_=y1h.rearrange("(w p) f -> p w f", p=128))

    state = {"prev": st2, "cur": st0, "scat": st1}

    def shard_to_yfull(t):
        ag_in = dram.tile([g.rows_core, H], F32, tag="ag_in")
        y_full = dram.tile([g.nodes_pad, H], F32, addr_space="Shared",
                           tag="y_full")
        nc.sync.dma_start(
            out=ag_in[:].rearrange("(w p) f -> p w f", p=128), in_=t[:])
        nc.gpsimd.collective_compute(
            "AllGather", AF.bypass,
            replica_groups=[list(range(g.n_cores))],
            ins=[ag_in.opt()], outs=[y_full.opt()])
        return y_full

    def coef_ap(layer, k):
        return coefc[:, layer * 16 + k: layer * 16 + k + 1]

    def bias_bc(layer):
        a = biasc[:, layer * H:(layer + 1) * H]
        return bass.AP(a.tensor, a.offset,
                       [list(a.ap[0]), [0, g.win], list(a.ap[1])])

    def prop(layer, k, y_full):
        t_prev, t_cur, t_scat = state["prev"], state["cur"], state["scat"]
        scale = 1.0 if k == 1 else 2.0
        if k == 1:
            nc.vector.memset(t_scat[:], 0.0)
        else:
            nc.vector.tensor_scalar_mul(t_scat[:], t_prev[:], -1.0)
        def batch_body(s, b):
            zt = zpool.tile([128, g.bpc, H], F32, tag="zt")
            nc.gpsimd.dma_gather(
                out_ap=zt[:],
                in_ap=y_full[bass.ds(s * g.seg_rows, g.seg_rows), :],
                idxs_ap=colidx_sb[:, bass.ds(s, 1), bass.ds(b, 1), :],
                num_idxs=g.bpc * 128,
                num_idxs_reg=g.bpc * 128,
                elem_size=H,
                single_packet=False,
            )
            for cc in range(g.cells_batch):
                ps = cellps.tile([128, H], F32, tag="cellps")
                for j in range(g.cell):
                    c = cc * g.cell + j
                    st = spool.tile([128, 128], F32, tag="sm")
                    nc.vector.tensor_scalar(
                        out=st[:], in0=iota[:],
                        scalar1=rel32[:, bass.ds(s, 1), bass.ds(b, 1),
                                      bass.ds(c, 1)],
                        scalar2=val32[:, bass.ds(s, 1), bass.ds(b, 1),
                                      bass.ds(c, 1)],
                        op0=AF.is_equal, op1=AF.mult)
                    nc.tensor.matmul(
                        ps[:], lhsT=st[:], rhs=zt[:, c, :],
                        start=(j == 0), stop=(j == g.cell - 1))
                tgt = t_scat[:, bass.ds(b * g.cells_batch + cc, 1), :]
                nc.vector.scalar_tensor_tensor(
                    out=tgt, in0=ps[:], scalar=scale, in1=tgt,
                    op0=AF.mult, op1=AF.add)

        with tc.For_i(0, g.segs, staggered_reset=True) as s:
            tc.For_i_unrolled(0, g.n_batches, 1,
                              lambda b: batch_body(s, b), max_unroll=2)
        nc.vector.scalar_tensor_tensor(
            out=acc[:], in0=t_scat[:], scalar=coef_ap(layer, k), in1=acc[:],
            op0=AF.mult, op1=AF.add)
        state["prev"], state["cur"], state["scat"] = t_cur, t_scat, t_prev

    for layer in range(g.n_layers):
        if layer > 0:
            nc.vector.tensor_tensor(
                out=acc[:], in0=acc[:], in1=bias_bc(layer - 1), op=AF.add)
            nc.vector.tensor_scalar_max(acc[:], acc[:], 0.0)
            t_new = state["scat"]
            for w in range(g.win):
                hT_ps = t1ps.tile([H, 128], F32, tag="t1ps")
                nc.tensor.transpose(hT_ps[:], acc[:, w, :], ident[:])
                hT = ltp.tile([H, 128], F32, tag="hT")
                nc.vector.tensor_copy(hT[:], hT_ps[:])
                yT_ps = t1ps.tile([H, 128], F32, tag="t1ps")
                nc.tensor.matmul(
                    yT_ps[:], lhsT=wts[:, (layer - 1) * H:layer * H],
                    rhs=hT[:], start=True, stop=True)
                yT = ltp.tile([H, 128], F32, tag="hT")
                nc.vector.tensor_copy(yT[:], yT_ps[:])
                y_ps = t2ps.tile([128, H], F32, tag="t2ps")
                nc.tensor.transpose(y_ps[:], yT[:], ident[:H, :H])
                nc.vector.tensor_copy(t_new[:, w, :], y_ps[:])
            state["scat"] = state["cur"]
            state["cur"] = t_new
        y_full = shard_to_yfull(state["cur"])
        nc.vector.tensor_scalar(
            out=acc[:], in0=state["cur"][:], scalar1=coef_ap(layer, 0),
            scalar2=None, op0=AF.mult)
        for k in range(1, g.kcheb):
            prop(layer, k, y_full)
            if k < g.kcheb - 1:
                y_full = shard_to_yfull(state["cur"])

    nc.vector.tensor_tensor(
        out=acc[:], in0=acc[:], in1=bias_bc(g.n_layers - 1), op=AF.add)
    nc.vector.tensor_scalar_max(acc[:], acc[:], 0.0)

    pool_ps = poolps.tile([128, H], F32)
    for w in range(g.win):
        pt = spool.tile([128, 128], F32, tag="pt")
        nc.vector.tensor_scalar(
            out=pt[:], in0=iota[:], scalar1=brel32[:, w:w + 1], scalar2=None,
            op0=AF.is_equal)
        nc.tensor.matmul(pool_ps[:], lhsT=pt[:], rhs=acc[:, w, :],
                         start=(w == 0), stop=(w == g.win - 1))
    pool_sb = sb.tile([128, H], F32)
    nc.vector.tensor_copy(pool_sb[:], pool_ps[:])
    nc.sync.dma_start(pool_out[:], pool_sb[:])


class _SpmdRunner:
    def __init__(self, nc, n_cores):
        import jax
        from jax.sharding import Mesh, PartitionSpec
        from jax.experimental.shard_map import shard_map
        from concourse.bass2jax import (
            _bass_exec_p, install_neuronx_cc_hook, partition_id_tensor)
        import concourse.mybir as mybir

        install_neuronx_cc_hook()
        self.n_cores = n_cores
        part_name = (nc.partition_id_tensor.name
                     if nc.partition_id_tensor is not None else None)
        in_names, out_names, out_avals, zero_outs = [], [], [], []
        for alloc in nc.m.functions[0].allocations:
            if not isinstance(alloc, mybir.MemoryLocationSet):
                continue
            name = alloc.memorylocations[0].name
            if alloc.kind == "ExternalInput":
                if name != part_name:
                    in_names.append(name)
            elif alloc.kind == "ExternalOutput":
                aval = jax.core.ShapedArray(
                    tuple(alloc.tensor_shape), mybir.dt.np(alloc.dtype))
                out_names.append(name)
                out_avals.append(aval)
                zero_outs.append(np.zeros(aval.shape, aval.dtype))
        self.n_params = len(in_names)
        self.in_names = list(in_names)
        self.out_names = list(out_names)
        self.out_avals = out_avals
        self.zero_outs = zero_outs
        all_in_names = in_names + out_names
        if part_name is not None:
            all_in_names = all_in_names + [part_name]

        def _body(*args):
            operands = list(args)
            if part_name is not None:
                operands.append(partition_id_tensor())
            return tuple(_bass_exec_p.bind(
                *operands,
                out_avals=tuple(out_avals),
                in_names=tuple(all_in_names),
                out_names=tuple(out_names),
                lowering_input_output_aliases=(),
                sim_require_finite=True,
                sim_require_nnan=True,
                nc=nc,
            ))

        devices = jax.devices()[:n_cores]
        self.mesh = Mesh(np.asarray(devices), ("core",))
        n_outs = len(out_names)
        donate = tuple(range(self.n_params, self.n_params + n_outs))
        self.fn = jax.jit(
            shard_map(_body, mesh=self.mesh,
                      in_specs=(PartitionSpec("core"),) * (self.n_params + n_outs),
                      out_specs=(PartitionSpec("core"),) * n_outs,
                      check_rep=False),
            donate_argnums=donate, keep_unused=True)

    def run(self, concat_by_name):
        concat_in = [concat_by_name[n] for n in self.in_names]
        zeros = [np.zeros((self.n_cores * z.shape[0], *z.shape[1:]), z.dtype)
                 for z in self.zero_outs]
        out_arrs = self.fn(*concat_in, *zeros)
        return {n: np.asarray(out_arrs[i]).reshape(
                    self.n_cores, *self.out_avals[i].shape)
                for i, n in enumerate(self.out_names)}


_GEOM = Geom()
_RUNNER = None
_IMPORT_ERR = None


def _init():
    global _RUNNER, _IMPORT_ERR
    try:
        import concourse.bacc as bacc
        import concourse.mybir as mybir
        import concourse.tile as tile
        g = _GEOM
        nc = bacc.Bacc("TRN2", target_bir_lowering=False, debug=False,
                       num_devices=g.n_cores)
        specs = _input_specs(g)
        ins = {name: nc.dram_tensor(name, shape,
                                    mybir.dt.from_np(np.dtype(dt)),
                                    kind="ExternalInput").ap()
               for name, (shape, dt) in specs.items()}
        outs = {"pool": nc.dram_tensor("pool", [128, g.hid], mybir.dt.float32,
                                       kind="ExternalOutput").ap()}
        with tile.TileContext(nc) as tc:
            with ExitStack() as ctx:
                _build_gnn(ctx, tc, outs, ins, g)
        nc.compile()
        runner = _SpmdRunner(nc, g.n_cores)
        # warm-up with the same arg types as the real call (y1h pre-put
        # as a sharded device array, the rest np): triggers trace + NEFF
        # compile + device load.
        import jax
        from jax.sharding import NamedSharding, PartitionSpec
        sh = NamedSharding(runner.mesh, PartitionSpec("core"))
        runner.sharding = sh
        dummy = {name: np.zeros((g.n_cores * s[0], *s[1:]), np.dtype(dt))
                 for name, (s, dt) in specs.items()}
        dummy["y1h"] = jax.device_put(dummy["y1h"], sh)
        runner.run(dummy)
        _RUNNER = runner
    except Exception as e:  # fall back to CPU path at call time
        _IMPORT_ERR = e


def _pack_concat(g: Geom, x, edge_index, batch, W1, theta1, b1, Ws, thetas,
                 bs):
    """Concat-layout (axis0 = core-major) input arrays, or None if the input
    does not fit the fixed grid."""
    row = np.ascontiguousarray(edge_index[0]).astype(np.int32)
    col = np.ascontiguousarray(edge_index[1]).astype(np.int32)
    n, e = x.shape[0], row.shape[0]

    deg = np.bincount(row, minlength=n).astype(np.float32)
    dinv = 1.0 / np.sqrt(np.maximum(deg, 1.0))
    val = -(dinv[row] * dinv[col])

    n_win_g = g.win * g.n_cores
    if (n > g.nodes_pad) or (row.max(initial=0) >> 7) >= n_win_g:
        return None
    key = ((row >> 7) * np.int32(g.segs) + col // np.int32(g.seg_rows))
    counts = np.bincount(key, minlength=n_win_g * g.segs)
    if counts.max() > g.cell * 128:
        return None
    order = np.argsort(key.astype(np.uint16), kind="stable")
    k_sorted = key[order]
    starts = np.zeros(n_win_g * g.segs, np.int32)
    np.cumsum(counts[:-1], dtype=np.int32, out=starts[1:])
    pos = np.arange(e, dtype=np.int32) - starts[k_sorted]

    wg = k_sorted // g.segs
    sg = k_sorted % g.segs
    core = wg // g.win
    wl = wg % g.win
    chunk_in_seg = wl * np.int32(g.cell) + (pos >> 7)
    slot = ((sg * np.int32(g.chunks_seg) + chunk_in_seg) << 7) + (pos & 127)
    flat = core * np.int32(g.segs * g.chunks_seg * 128) + slot

    tot = g.n_cores * g.segs * g.chunks_seg * 128
    col16 = np.zeros(tot, np.int16)
    rel8 = np.zeros(tot, np.uint8)
    val16 = np.zeros(tot, np.float16)
    col16[flat] = (col[order] - sg * np.int32(g.seg_rows)).astype(np.int16)
    rel8[flat] = (row[order] & 127).astype(np.uint8)
    val16[flat] = val[order].astype(np.float16)

    nb, bpc = g.n_batches, g.bpc
    i_idx = np.arange(bpc * 128)
    colidx = np.zeros((g.n_cores, g.segs, nb, 16, bpc * 8), np.int16)
    colidx[:, :, :, i_idx % 16, i_idx // 16] = col16.reshape(
        g.n_cores, g.segs, nb, bpc * 128)
    colidx = np.ascontiguousarray(colidx.transpose(0, 3, 1, 2, 4)).reshape(
        g.n_cores * 16, g.segs, nb, bpc * 8)
    rel8 = np.ascontiguousarray(np.moveaxis(
        rel8.reshape(g.n_cores, g.segs, nb, bpc, 128), -1, 1)).reshape(
        g.n_cores * 128, g.segs, nb, bpc)
    val16 = np.ascontiguousarray(np.moveaxis(
        val16.reshape(g.n_cores, g.segs, nb, bpc, 128), -1, 1)).reshape(
        g.n_cores * 128, g.segs, nb, bpc)

    bat = np.full(g.nodes_pad, -1.0, np.float32)
    bat[:n] = batch.astype(np.float32)
    brel = np.ascontiguousarray(np.swapaxes(
        bat.reshape(g.n_cores, g.win, 128), 1, 2)).astype(np.float16).reshape(
        g.n_cores * 128, g.win)

    H = g.hid
    wts = np.ascontiguousarray(
        np.moveaxis(Ws.astype(np.float32), 0, 1)).reshape(H, -1)
    wts = np.tile(wts, (g.n_cores, 1))
    biasc = np.concatenate(
        [np.asarray(b1, np.float32).reshape(1, H)] +
        [np.asarray(bs[i], np.float32).reshape(1, H)
         for i in range(g.n_layers - 1)], axis=1)
    biasc = np.tile(np.broadcast_to(biasc, (128, g.n_layers * H)),
                    (g.n_cores, 1))
    coef = np.zeros((g.n_layers, 16), np.float32)
    coef[0, :g.kcheb] = np.asarray(theta1, np.float32).mean(axis=0)
    for i in range(g.n_layers - 1):
        coef[i + 1, :g.kcheb] = np.asarray(thetas[i], np.float32).mean(axis=0)
    coefc = np.tile(np.broadcast_to(coef.reshape(1, -1),
                                    (128, g.n_layers * 16)), (g.n_cores, 1))

    return {"colidx": colidx, "relv": rel8, "valv": val16,
            "batchrel": brel, "wts": wts, "biasc": biasc, "coefc": coefc}


def _pack_y1(g: Geom, x, W1):
    n = x.shape[0]
    y1 = np.asarray(x, np.float32) @ np.asarray(W1, np.float32)
    y1p = np.zeros((g.nodes_pad, g.hid), np.float16)
    y1p[:n] = y1.astype(np.float16)
    return y1p


def _epilogue(g: Geom, pool_parts, batch, lin1_w, lin1_b, lin2_w, lin2_b):
    sums = pool_parts.sum(axis=0)[:g.n_graphs]
    cnt = np.bincount(batch.astype(np.int64),
                      minlength=g.n_graphs).astype(np.float32)
    pooled = sums / np.maximum(cnt, 1.0)[:, None]
    gout = np.maximum(pooled @ lin1_w + lin1_b, 0.0)
    logits = gout @ lin2_w + lin2_b
    m = logits.max(axis=1, keepdims=True)
    out = logits - m - np.log(np.exp(logits - m).sum(axis=1))[:, None]
    return out.astype(np.float32)


def _kernel_cpu(x, edge_index, batch, W1, theta1, b1, Ws, thetas, bs,
                lin1_w, lin1_b, lin2_w, lin2_b):
    """scipy fallback (the previous baseline)."""
    import scipy.sparse as sp
    x = np.asarray(x, np.float32)
    row = np.asarray(edge_index[0]).astype(np.int64)
    col = np.asarray(edge_index[1]).astype(np.int64)
    n = x.shape[0]
    deg = np.bincount(row, minlength=n).astype(np.float32)
    dinv = 1.0 / np.sqrt(np.maximum(deg, 1.0))
    vals = (-dinv[row] * dinv[col]).astype(np.float32)
    A = sp.csr_matrix((vals, (row, col)), shape=(n, n))

    def spectral_layer(h, W, theta, b):
        y = h @ np.asarray(W, np.float32)
        coeff = np.asarray(theta, np.float32).mean(axis=0)
        t_prev, t_cur = y, A @ y
        out = coeff[0] * t_prev + coeff[1] * t_cur
        for k in range(2, K):
            t_next = 2.0 * (A @ t_cur) - t_prev
            out = out + coeff[k] * t_next
            t_prev, t_cur = t_cur, t_next
        return out + np.asarray(b, np.float32)

    h = np.maximum(spectral_layer(x, W1, theta1, b1), 0.0)
    for i in range(NUM_LAYERS - 1):
        h = np.maximum(spectral_layer(h, Ws[i], thetas[i], bs[i]), 0.0)
    sums = np.zeros((NUM_GRAPHS, HID), np.float32)
    np.add.at(sums, np.asarray(batch, np.int64), h)
    cnt = np.bincount(np.asarray(batch, np.int64),
                      minlength=NUM_GRAPHS).astype(np.float32)
    pooled = sums / np.maximum(cnt, 1.0)[:, None]
    g = np.maximum(pooled @ np.asarray(lin1_w, np.float32) + lin1_b, 0.0)
    logits = g @ np.asarray(lin2_w, np.float32) + lin2_b
    m = logits.max(axis=1, keepdims=True)
    out = logits - m - np.log(np.exp(logits - m).sum(axis=1))[:, None]
    return out.astype(np.float32)


def kernel(x, edge_index, batch, W1, theta1, b1, Ws, thetas, bs,
           lin1_w, lin1_b, lin2_w, lin2_b):
    try:
        if _RUNNER is None:
            raise RuntimeError(f"no trn2 runner: {_IMPORT_ERR}")
        g = _GEOM
        x = np.asarray(x)
        if x.shape != (N, F_IN):
            raise RuntimeError("unexpected shape")
        import jax
        y1p = _pack_y1(g, x, W1)
        y1_dev = jax.device_put(y1p, _RUNNER.sharding)  # async upload
        packed = _pack_concat(g, x, np.asarray(edge_index), np.asarray(batch),
                              np.asarray(W1), np.asarray(theta1),
                              np.asarray(b1), np.asarray(Ws),
                              np.asarray(thetas), np.asarray(bs))
        if packed is None:
            raise RuntimeError("grid capacity exceeded")
        packed["y1h"] = y1_dev
        res = _RUNNER.run(packed)
        pool_parts = res["pool"]
        if not np.isfinite(pool_parts).all():
            raise RuntimeError("non-finite device result")
        return _epilogue(g, pool_parts, np.asarray(batch),
                         np.asarray(lin1_w, np.float32),
                         np.asarray(lin1_b, np.float32),
                         np.asarray(lin2_w, np.float32),
                         np.asarray(lin2_b, np.float32))
    except Exception:
        return _kernel_cpu(x, edge_index, batch, W1, theta1, b1, Ws, thetas,
                           bs, lin1_w, lin1_b, lin2_w, lin2_b)


_init()
